# revision 7
# baseline (speedup 1.0000x reference)
"""Trainium2 Bass kernel for a 2-layer GCN (PyG GCNConv semantics) on 8 NeuronCores.

Strategy (dst-sharding, per the sharding hint):
  - nodes sharded 12500/core (padded to 12544 = 98*128 tiles of 128 rows)
  - edges partitioned by destination core; within a core grouped by
    (dst-tile, src-group) and padded to 128-edge chunks
  - per chunk: dma_gather of 128 source rows (fp16) + a one-hot matrix
    P[t,r] = (iota==dst_rel)*dinv[dst] built with one fused tensor_scalar,
    then a PE matmul accumulates agg^T[d,r] into PSUM
  - per dst-tile: agg^T @ W^T flips the orientation back to [row, feat]
  - norm separability: norm_e = dinv[src]*dinv[dst]; the dinv[src] factor is
    pre-scaled into the gather source rows, dinv[dst] rides inside P
  - layer 1 -> AllGather of y1 = dinv*(h2) shards -> layer 2
"""
import sys

sys.path.insert(0, "/opt/trn_rl_repo")

import numpy as np

N = 100000
E = 1600000
D = 128
CORES = 8
S = 12500          # real nodes per core
TPC = 98           # dst tiles per core
SP = TPC * 128     # padded nodes per core (12544)
NP = CORES * SP    # padded global rows (100352)
GRP = 32768        # src-group width (int16 gather-index limit)
NGRP = 4
BLK = 8            # dst tiles per gather block
GMAX = 8192        # max indices per dma_gather instruction (ring capacity)


def _build_schedule(src, dst):
    """Static chunk schedule shared by all cores (SPMD: one instruction
    stream). Returns per-core slot arrays + the chunk/block layout."""
    core = dst // S
    dl = dst % S
    t = dl >> 7
    r = dl & 127
    sp = (src // S) * SP + (src % S)
    g = np.minimum(sp // GRP, NGRP - 1)
    srel = sp - g * GRP

    key = (core * TPC + t) * NGRP + g
    order = np.argsort(key, kind="stable")
    cnt = np.bincount(key, minlength=CORES * TPC * NGRP).reshape(CORES, TPC, NGRP)
    K = -(-cnt.max(0) // 128)  # [TPC, NGRP] chunks per (tile, group)

    # chunk layout order: for b in blocks: for g: for t in b: for k in K[t,g]
    chunk_start = np.zeros((TPC, NGRP), np.int64)
    blocks = []
    nchunks = 0
    for b in range((TPC + BLK - 1) // BLK):
        tiles = list(range(b * BLK, min((b + 1) * BLK, TPC)))
        col = 0
        gathers = []
        tile_chunks = {tt: [] for tt in tiles}
        for gg in range(NGRP):
            c0 = col
            slot0 = nchunks * 128
            for tt in tiles:
                chunk_start[tt, gg] = nchunks
                for _ in range(int(K[tt, gg])):
                    tile_chunks[tt].append((col, nchunks))
                    col += 1
                    nchunks += 1
            # split into <=8192-index instructions: the SWDGE descriptor
            # ring holds 1024 descriptors/direction and a gather needs
            # num_idxs/16+1 — a single too-big instruction deadlocks HW
            c_at = c0
            s_at = slot0
            while c_at < col:
                ncols = min(col - c_at, GMAX // 128)
                gathers.append((gg, c_at, c_at + ncols, s_at, ncols * 128))
                c_at += ncols
                s_at += ncols * 128
        blocks.append(dict(tiles=tiles, C=col, gathers=gathers, chunks=tile_chunks))
    NC = nchunks
    NSLOT = NC * 128

    # per-core slot arrays
    skey = key[order]
    runs = np.flatnonzero(np.diff(skey)) + 1
    starts = np.r_[0, runs]
    lens = np.diff(np.r_[starts, len(skey)])
    pos = np.arange(len(skey)) - np.repeat(starts, lens)
    slot = chunk_start[t[order], g[order]] * 128 + pos

    idx_slot = np.zeros((CORES, NSLOT), np.int16)
    dr_slot = np.full((CORES, NSLOT), -1.0, np.float32)
    wd_slot = np.zeros((CORES, NSLOT), np.float32)
    co = core[order]
    idx_slot[co, slot] = srel[order].astype(np.int16)
    dr_slot[co, slot] = r[order].astype(np.float32)
    return idx_slot, dr_slot, wd_slot, order, co, slot, blocks, NC, NSLOT


def _build_bass(blocks, NC, NSLOT):
    import concourse.bacc as bacc
    import concourse.tile as tile
    import concourse.mybir as mybir

    dt = mybir.dt
    nc = bacc.Bacc("TRN2", target_bir_lowering=False, debug=False, num_devices=CORES)

    xs_in = nc.dram_tensor("xs", [NP, D], dt.float16, kind="ExternalInput")
    w1t_in = nc.dram_tensor("w1t", [D, D], dt.float16, kind="ExternalInput")
    w2t_in = nc.dram_tensor("w2t", [D, D], dt.float16, kind="ExternalInput")
    iota_in = nc.dram_tensor("iota", [128, 128], dt.float16, kind="ExternalInput")
    idx_in = nc.dram_tensor("idx", [128, NSLOT // 16], dt.int16, kind="ExternalInput")
    dr_in = nc.dram_tensor("dr", [128, NC], dt.float32, kind="ExternalInput")
    wd_in = nc.dram_tensor("wd", [128, NC], dt.float32, kind="ExternalInput")
    dinv_in = nc.dram_tensor("dinvcol", [128, TPC], dt.float32, kind="ExternalInput")
    c1d_in = nc.dram_tensor("c1d", [SP, D], dt.float16, kind="ExternalInput")
    c2_in = nc.dram_tensor("c2", [SP, D], dt.float32, kind="ExternalInput")
    out_ext = nc.dram_tensor("out", [SP, D], dt.float32, kind="ExternalOutput")

    GBASE = [i * GRP for i in range(NGRP)]
    GLEN = [min(GRP, NP - i * GRP) for i in range(NGRP)]

    with tile.TileContext(nc) as tc:
        with (
            tc.tile_pool(name="const", bufs=1) as cpool,
            tc.tile_pool(name="mblk", bufs=2) as mpool,
            tc.tile_pool(name="pbuf", bufs=6) as ppool,
            tc.tile_pool(name="gs", bufs=4) as gspool,
            tc.tile_pool(name="ytmp", bufs=4) as ytpool,
            tc.tile_pool(name="cload", bufs=4) as clpool,
            tc.tile_pool(name="psumG", bufs=4, space="PSUM") as pgpool,
            tc.tile_pool(name="psumH", bufs=4, space="PSUM") as phpool,
            tc.tile_pool(name="dram", bufs=1, space="DRAM") as dram_pool,
        ):
            iota_t = cpool.tile([128, 128], dt.float16)
            nc.sync.dma_start(out=iota_t[:], in_=iota_in[:, :])
            w1t_t = cpool.tile([D, D], dt.float16)
            nc.sync.dma_start(out=w1t_t[:], in_=w1t_in[:, :])
            w2t_t = cpool.tile([D, D], dt.float16)
            nc.sync.dma_start(out=w2t_t[:], in_=w2t_in[:, :])
            idx_t = cpool.tile([128, NSLOT // 16], dt.int16)
            nc.sync.dma_start(out=idx_t[:], in_=idx_in[:, :])
            dr_t = cpool.tile([128, NC], dt.float32)
            nc.sync.dma_start(out=dr_t[:], in_=dr_in[:, :])
            wd_t = cpool.tile([128, NC], dt.float32)
            nc.sync.dma_start(out=wd_t[:], in_=wd_in[:, :])
            dinv_t = cpool.tile([128, TPC], dt.float32)
            nc.sync.dma_start(out=dinv_t[:], in_=dinv_in[:, :])

            y1_shard = dram_pool.tile([SP, D], dt.float16)
            y1_full = dram_pool.tile([NP, D], dt.float16)

            def layer(src_dram, wt_t, last):
                for blk in blocks:
                    C = blk["C"]
                    m_t = mpool.tile([128, C, D], dt.float16, tag="m")
                    for gg, c0, c1, slot0, num in blk["gathers"]:
                        nc.gpsimd.dma_gather(
                            m_t[:, c0:c1, :],
                            src_dram[GBASE[gg] : GBASE[gg] + GLEN[gg], :],
                            idx_t[:, slot0 // 16 : (slot0 + num) // 16],
                            num,
                            num,
                            D,
                            single_packet=False,
                        )
                    for tt in blk["tiles"]:
                        chunks = blk["chunks"][tt]
                        psum_g = pgpool.tile([128, 128], dt.float32, space="PSUM")
                        nchk = len(chunks)
                        for i, (col, chid) in enumerate(chunks):
                            p_t = ppool.tile([128, 128], dt.float16, tag="p")
                            nc.any.tensor_scalar(
                                out=p_t[:],
                                in0=iota_t[:],
                                scalar1=dr_t[:, chid : chid + 1],
                                scalar2=wd_t[:, chid : chid + 1],
                                op0=mybir.AluOpType.is_equal,
                                op1=mybir.AluOpType.mult,
                            )
                            nc.tensor.matmul(
                                psum_g[:],
                                lhsT=m_t[:, col, :],
                                rhs=p_t[:],
                                start=(i == 0),
                                stop=(i == nchk - 1),
                            )
                        gs_t = gspool.tile([128, 128], dt.float16, tag="gs")
                        nc.any.tensor_copy(out=gs_t[:], in_=psum_g[:])
                        psum_h = phpool.tile([128, 128], dt.float32, space="PSUM")
                        nc.tensor.matmul(
                            psum_h[:], lhsT=gs_t[:], rhs=wt_t[:], start=True, stop=True
                        )
                        rows = slice(tt * 128, (tt + 1) * 128)
                        if not last:
                            tmp_t = ytpool.tile([128, 128], dt.float16, tag="yt")
                            nc.any.tensor_scalar(
                                out=tmp_t[:],
                                in0=psum_h[:],
                                scalar1=dinv_t[:, tt : tt + 1],
                                scalar2=None,
                                op0=mybir.AluOpType.mult,
                            )
                            c1_t = clpool.tile([128, 128], dt.float16, tag="c1")
                            nc.sync.dma_start(out=c1_t[:], in_=c1d_in[rows, :])
                            y1_t = ytpool.tile([128, 128], dt.float16, tag="y1")
                            nc.any.tensor_tensor(
                                out=y1_t[:],
                                in0=tmp_t[:],
                                in1=c1_t[:],
                                op=mybir.AluOpType.add,
                            )
                            nc.sync.dma_start(out=y1_shard[rows, :], in_=y1_t[:])
                        else:
                            c2_t = clpool.tile([128, 128], dt.float32, tag="c2")
                            nc.sync.dma_start(out=c2_t[:], in_=c2_in[rows, :])
                            o_t = ytpool.tile([128, 128], dt.float32, tag="o")
                            nc.any.tensor_tensor(
                                out=o_t[:],
                                in0=psum_h[:],
                                in1=c2_t[:],
                                op=mybir.AluOpType.add,
                            )
                            nc.sync.dma_start(out=out_ext[rows, :], in_=o_t[:])

            layer(xs_in, w1t_t, last=False)
            nc.gpsimd.collective_compute(
                "AllGather",
                mybir.AluOpType.bypass,
                replica_groups=[list(range(CORES))],
                ins=[y1_shard.opt()],
                outs=[y1_full.opt()],
            )
            layer(y1_full, w2t_t, last=True)

    nc.compile()
    return nc


def _prepare(x, edge_index, perturb_first, perturb_last, W1, b1, W2, b2):
    x = np.asarray(x, np.float32)
    edge_index = np.asarray(edge_index)
    src = np.concatenate([edge_index[0], np.arange(N, dtype=edge_index.dtype)]).astype(
        np.int64
    )
    dst = np.concatenate([edge_index[1], np.arange(N, dtype=edge_index.dtype)]).astype(
        np.int64
    )
    deg = np.bincount(dst, minlength=N).astype(np.float32)
    dinv = 1.0 / np.sqrt(deg)

    idx_slot, dr_slot, wd_slot, order, co, slot, blocks, NC, NSLOT = _build_schedule(
        src, dst
    )
    wd_slot[co, slot] = dinv[dst[order]]

    # gather source: xs = dinv * x, padded to NP rows in shard-major layout
    xs = np.zeros((NP, D), np.float16)
    dinv_x = (dinv[:, None] * x).astype(np.float16)
    for c in range(CORES):
        xs[c * SP : c * SP + S] = dinv_x[c * S : (c + 1) * S]

    iota = np.broadcast_to(np.arange(128, dtype=np.float16), (128, 128)).copy()
    w1t = np.asarray(W1, np.float32).T.astype(np.float16).copy()
    w2t = np.asarray(W2, np.float32).T.astype(np.float16).copy()

    c1 = np.asarray(perturb_first, np.float32) + np.asarray(b1, np.float32)[None, :]
    c1d = dinv[:, None] * c1
    c2 = np.asarray(perturb_last, np.float32) + np.asarray(b2, np.float32)[None, :]

    in_maps = []
    for c in range(CORES):
        rows = slice(c * S, (c + 1) * S)
        c1d_p = np.zeros((SP, D), np.float16)
        c1d_p[:S] = c1d[rows].astype(np.float16)
        c2_p = np.zeros((SP, D), np.float32)
        c2_p[:S] = c2[rows]
        dinvcol = np.zeros((TPC * 128,), np.float32)
        dinvcol[:S] = dinv[rows]
        idx_l = np.tile(idx_slot[c].reshape(-1, 16).T, (8, 1)).copy()
        dr_l = np.ascontiguousarray(dr_slot[c].reshape(NC, 128).T)
        wd_l = np.ascontiguousarray(wd_slot[c].reshape(NC, 128).T)
        in_maps.append(
            {
                "xs": xs,
                "w1t": w1t,
                "w2t": w2t,
                "iota": iota,
                "idx": idx_l,
                "dr": dr_l,
                "wd": wd_l,
                "dinvcol": np.ascontiguousarray(dinvcol.reshape(TPC, 128).T),
                "c1d": c1d_p,
                "c2": c2_p,
            }
        )
    return in_maps, blocks, NC, NSLOT


def kernel(x, edge_index, perturb_first, perturb_last, W1, b1, W2, b2, _results=[]):
    from concourse.bass_utils import run_bass_kernel_spmd

    in_maps, blocks, NC, NSLOT = _prepare(
        x, edge_index, perturb_first, perturb_last, W1, b1, W2, b2
    )
    nc = _build_bass(blocks, NC, NSLOT)
    res = run_bass_kernel_spmd(nc, in_maps, core_ids=list(range(CORES)))
    _results.append(res)
    out = np.concatenate([res.results[c]["out"][:S] for c in range(CORES)], axis=0)
    return out.astype(np.float32)
